# revision 37
# baseline (speedup 1.0000x reference)
"""Graphormer multi-head attention on 8 trn2 NeuronCores.

Sharding: sequence-parallel over the 8 sorted batch segments (one graph
per core). Each core runs dense block attention for all 8 heads over its
~512-node segment, padded to a common NB so the program is SPMD.

I/O design: the per-call cost in this environment is dominated by input
marshaling (per-buffer and per-byte), so the program takes ONE packed
fp16 input per core (~1MB: x^T slice, augmented projection weights,
bucketed edge-bias COO data, column mask) and returns one fp16 output.
The edge bias is scattered on-device: scaled one-hot matrices are built
with iota + tensor_scalar(is_equal, mult) and injected into the score
PSUM accumulation as extra rank-|bucket| matmuls.

Math (transposed so the softmax reduction rides the matmul contraction):
  S^T[c, r] = K[c, :] . Q[r, :] / sqrt(HD)     (PE, fp16 operands)
  S^T      += sum_e onehotC_e(c) eb_e onehotR_e(r)   (PE, per c-chunk bucket)
  P = exp(S^T + colmask - 6)                   (ACT; -6 keeps exp in fp16)
  OT'[d, r] = sum_c V'[c, d] P[c, r]           (PE; V' has a ones column ->
                                                row 32 of OT' = denominator)
  outT = OT'[0:32] * bcast(1/den)              (DVE; bcast via K=1 outer product)
  y^T = Wo'^T @ [outT; 1]                      (PE; bias via augmented ones row)
"""

import sys

for _p in ("/opt/trn_rl_repo",):
    if _p not in sys.path:
        sys.path.insert(0, _p)

import numpy as np

import concourse.bass as bass
import concourse.mybir as mybir
import concourse.tile as tile
from concourse.bass_utils import run_bass_kernel_spmd

N, D, H, HD, NCORES = 4096, 256, 8, 32, 8

# ---------------------------------------------------------------------------
# This toolchain's CoreV3 codegen accepts at most ONE semaphore wait per
# engine instruction ("Too many sync wait commands").  Tile freely emits
# several.  Engine queues execute in order, so it is equivalent to hoist all
# but one wait onto single-wait NoOps inserted immediately before the
# instruction on the same engine.  Do that as a BIR-JSON rewrite just before
# neuronxcc compilation.
import json as _json

import concourse.bass2jax as _b2j

_SKIP_OPS = {"EventSemaphore", "UnconditionalBranch", "ConditionalBranch"}


def _split_multiwaits(bir_json: bytes) -> bytes:
    d = _json.loads(bir_json)
    nid = [0]
    for fn in d.get("functions", []):
        for blk in fn.get("blocks", []):
            out = []
            for inst in blk.get("instructions", []):
                si = inst.get("sync_info")
                ow = (si or {}).get("on_wait") or []
                if len(ow) > 1 and inst.get("opcode") not in _SKIP_OPS:
                    for w in ow[:-1]:
                        nid[0] += 1
                        out.append(
                            {
                                "debug": inst.get("debug", 0),
                                "engine": inst["engine"],
                                "ins": [],
                                "name": f"I-waitsplit-{nid[0]}",
                                "opcode": "NoOp",
                                "outs": [],
                                "sync_info": {"on_update": [], "on_wait": [w]},
                            }
                        )
                    si["on_wait"] = [ow[-1]]
                out.append(inst)
            blk["instructions"] = out
    return _json.dumps(d).encode()


_orig_cbk = _b2j.compile_bir_kernel


def _cbk(bir_json, tmpdir, neff_name="file.neff"):
    return _orig_cbk(_split_multiwaits(bir_json), tmpdir, neff_name=neff_name)


if getattr(_b2j.compile_bir_kernel, "__name__", "") != "_cbk":
    _b2j.compile_bir_kernel = _cbk

SCALE = 1.0 / np.sqrt(HD)
MSK_VALID = -6.0  # constant shift: cancels in softmax, keeps exp(.) in fp16
MSK_PAD = -60000.0  # exp -> 0 for padded key columns
NR = 4  # query-row ranges per bucket (scatter matmuls stream only their range)
SLOTS_PER_BLK = 12  # (cc, range) slots of 10 cols per 128-wide edge block

_prog_cache = {}
_last_in_maps = None


def _ranges_for(NB):
    # row ranges that never cross a 512-wide PSUM bank boundary: three
    # 32-aligned ranges over [0, 512), then one per 512 chunk beyond
    out = [(0, 160), (160, 176), (336, 176)]
    s = 512
    while s < NB:
        out.append((s, min(512, NB - s)))
        s += 512
    return out


def _build_program(NB, CAP, HB):
    NCH = NB // 128
    ECH = CAP // 128
    splits = [(s, min(512, NB - s)) for s in range(0, NB, 512)]
    ranges = _ranges_for(NB)
    NRR = len(ranges)
    NBLK = -(-(NCH * NRR) // SLOTS_PER_BLK)
    NKI = 3 if HB else 2  # drop the rank-1 bias aug chunk when biases are 0

    def subsplits(col0, wn):
        # intersect a row range with the 512-wide PSUM bank splits
        out = []
        for fs0, fsn in splits:
            a, bnd = max(col0, fs0), min(col0 + wn, fs0 + fsn)
            if a < bnd:
                out.append((a, bnd - a))
        return out

    f32 = mybir.dt.float32
    f16 = mybir.dt.float16

    # packed input row offsets (rows of 128 fp16)
    XT0 = 0
    W0 = XT0 + 256 * NCH
    E0 = W0 + 4 * 514
    M0 = E0 + NBLK * CAP
    RTOT = M0 + 128

    nc = bass.Bass()
    inp_d = nc.declare_dram_parameter("inp", [RTOT, 128], f16, isOutput=False)
    yt_d = nc.declare_dram_parameter("yt", [256, NB], f16, isOutput=True)

    kch = [(0, 128), (128, 128), (256, 1)]  # contraction chunks of the 257-row aug

    with tile.TileContext(nc) as tc:
        with (
            tc.tile_pool(name="persist", bufs=1) as pp,
            tc.tile_pool(name="pexp", bufs=4) as pxp,
            tc.tile_pool(name="ps_s", bufs=2, space="PSUM") as sp,
            tc.tile_pool(name="ps_o", bufs=2, space="PSUM") as op,
        ):
            # ---- load persistent operands from the packed input ----
            # HWDGE (sync) carries the big x/weight loads; SWDGE (gpsimd)
            # carries the cast loads; scalar (HWDGE) the tiny bias rows.
            iota = pp.tile([128, NB], f16, tag="iota")
            nc.gpsimd.iota(
                iota[:],
                pattern=[[1, NB]],
                base=0,
                channel_multiplier=0,
                allow_small_or_imprecise_dtypes=True,
            )
            # iota repeated 8x along the head axis for the batched one-hot build
            iota8 = pp.tile([128, 8, 128], f16, tag="iota8")
            nc.gpsimd.iota(
                iota8[:],
                pattern=[[0, 8], [1, 128]],
                base=0,
                channel_multiplier=0,
                allow_small_or_imprecise_dtypes=True,
            )
            xt = []
            for kc in range(2):
                t = pp.tile([128, NB], f16, tag=f"xt{kc}", name=f"xt{kc}")
                nc.gpsimd.dma_start(
                    out=t[:],
                    in_=inp_d[
                        XT0 + kc * 128 * NCH : XT0 + (kc + 1) * 128 * NCH, :
                    ].rearrange("(p f) w -> p (f w)", f=NCH),
                )
                xt.append(t)
            ones16 = pp.tile([1, NB], f16, tag="ones16")
            nc.vector.memset(ones16[:], 1.0)
            xt.append(ones16)  # aug ones row, contraction chunk (256, 1)

            # PE warm-up: the HAM clock gate releases only after ~3.4us of
            # sustained matmul activity. While the weight DMAs land, run free
            # dummy matmuls so the real stream starts at full clock.
            warm = sp.tile([128, 512], f32, tag="s", name="warm")
            for _ in range(16):
                nc.tensor.matmul(
                    warm[:],
                    ones16[0:1, 0:128],
                    ones16[0:1, 0:512],
                    start=True,
                    stop=True,
                )

            # edge blocks first on the scalar queue so the one-hot builds can
            # start immediately: [128, ECH, 128] fp16 per block (indices
            # ≤ 2048 are exact in fp16, so comparisons stay exact)
            ed_blk = []
            for blk in range(NBLK):
                t = pp.tile([128, ECH, 128], f16, tag=f"ed{blk}")
                nc.sync.dma_start(
                    out=t[:],
                    in_=inp_d[E0 + blk * CAP : E0 + (blk + 1) * CAP, :].rearrange(
                        "(e p) w -> p e w", p=128
                    ),
                )
                ed_blk.append(t)
            wt = {}
            for wi, nm in enumerate(("wq", "wk", "wv", "wo")):
                base = W0 + wi * 514
                w2 = pp.tile([128, 2, 256], f16, tag=f"{nm}w2", name=f"{nm}w2")
                nc.sync.dma_start(
                    out=w2[:],
                    in_=inp_d[base : base + 512, :].rearrange(
                        "(kc p h) w -> p kc (h w)", kc=2, p=128, h=2
                    ),
                )
                wt[nm] = [w2[:, 0, :], w2[:, 1, :]]
                if HB:
                    wb = pp.tile([1, 256], f16, tag=f"{nm}wb", name=f"{nm}wb")
                    nc.scalar.dma_start(
                        out=wb[:],
                        in_=inp_d[base + 512 : base + 514, :].rearrange(
                            "(p h) w -> p (h w)", p=1
                        ),
                    )
                    wt[nm].append(wb[:])

            maskt = pp.tile([128, NCH], f16, tag="mask")
            nc.sync.dma_start(out=maskt[:], in_=inp_d[M0 : M0 + 128, 0:NCH])

            # f32 view of the index columns (is_equal requires an f32 scalar)
            ed32_blk = []
            for blk in range(NBLK):
                t = pp.tile([128, ECH, 128], f32, tag=f"ed32_{blk}")
                nc.vector.tensor_copy(t[:], ed_blk[blk][:])
                ed32_blk.append(t)

            def _slot(cc, j):
                slot = cc * NRR + j
                return slot // SLOTS_PER_BLK, (slot % SLOTS_PER_BLK) * 10

            def ed_cl(cc, j, ech):
                blk, w0 = _slot(cc, j)
                return ed32_blk[blk][:, ech, w0 : w0 + 1]

            def ed_rl(cc, j, ech):
                blk, w0 = _slot(cc, j)
                return ed32_blk[blk][:, ech, w0 + 1 : w0 + 2]

            def ed_eb(cc, j, ech):
                blk, w0 = _slot(cc, j)
                return ed_blk[blk][:, ech, w0 + 2 : w0 + 10]

            ones32 = pp.tile([1, 32], f32, tag="ones32")
            nc.vector.memset(ones32[:], 1.0)

            # ---- one-hot selectors per (key-chunk, row-range, edge chunk) --
            # ohr: row one-hots [edges, range] (only the range's columns);
            # sc8: per-head scaled col one-hots [edges, 8, 128]
            # = (iota == cl) * eb_h.  oh8 alternates DVE/GPSIMD for balance.
            ohr = {}
            sc8 = {}
            for cc in range(NCH):
                for j, (col0, wn) in enumerate(ranges):
                    for ech in range(ECH):
                        t = pp.tile([128, wn], f16, tag=f"ohr{cc}_{j}_{ech}")
                        nc.vector.tensor_scalar(
                            t[:],
                            iota[:, col0 : col0 + wn],
                            ed_rl(cc, j, ech),
                            None,
                            mybir.AluOpType.is_equal,
                        )
                        ohr[(cc, j, ech)] = t
                        oh8 = pxp.tile([128, 8, 128], f16, tag="oh8")
                        beng = nc.vector if (cc * NR + j) % 2 else nc.gpsimd
                        beng.tensor_scalar(
                            oh8[:],
                            iota8[:],
                            ed_cl(cc, j, ech),
                            None,
                            mybir.AluOpType.is_equal,
                        )
                        s8 = pp.tile([128, 8, 128], f16, tag=f"sc8{cc}_{j}_{ech}")
                        nc.vector.tensor_tensor(
                            s8[:],
                            oh8[:],
                            ed_eb(cc, j, ech).to_broadcast((128, 8, 128)),
                            mybir.AluOpType.mult,
                        )
                        sc8[(cc, j, ech)] = s8

            # ---- Q^T, K^T: two groups of 4 heads; head slice base partition
            # 0/32/64 auto, 96 via explicit tile_position.
            qk_tiles = {}
            for key in ("q", "k"):
                qk_tiles[key] = [
                    pp.tile([128, NB], f16, tag=f"{key}g{g}", name=f"{key}g{g}")
                    for g in range(2)
                ]

            def qk_slice(key, h):
                return qk_tiles[key][h // 4][(h % 4) * 32 : (h % 4) * 32 + 32]

            for nm, key, scl in (("wq", "q", SCALE), ("wk", "k", 1.0)):
                for mg in range(2):
                    acc = sp.tile([128, NB], f32, tag="s", name="acc")
                    for fs0, fsn in splits:
                        for ki in range(NKI):
                            nc.tensor.matmul(
                                acc[:, fs0 : fs0 + fsn],
                                wt[nm][ki][:, mg * 128 : (mg + 1) * 128],
                                xt[ki][:, fs0 : fs0 + fsn],
                                start=(ki == 0),
                                stop=(ki == NKI - 1),
                            )
                    nc.scalar.activation(
                        qk_tiles[key][mg][:],
                        acc[:],
                        mybir.ActivationFunctionType.Copy,
                        scale=scl,
                    )

            # ---- V natural layout, per 128-row chunk, with ones column ----
            v33 = []
            for rc in range(NCH):
                dst = pp.tile([128, 8, 33], f16, tag=f"v33_{rc}")
                acc = sp.tile([128, NB], f32, tag="s", name="accv")[:, 0:256].rearrange("p (h d) -> p h d", h=8)
                for ki in range(NKI):
                    nc.tensor.matmul(
                        acc[:],
                        xt[ki][:, rc * 128 : (rc + 1) * 128],
                        wt["wv"][ki][:],
                        start=(ki == 0),
                        stop=(ki == NKI - 1),
                    )
                nc.scalar.activation(
                    dst[:, :, 0:32], acc[:], mybir.ActivationFunctionType.Copy
                )
                nc.vector.memset(dst[:, :, 32:33], 1.0)
                v33.append(dst)

            # ---- attention per head ----
            outT = [
                pp.tile([128, NB], f16, tag=f"outT{mg}", name=f"outT{mg}")
                for mg in range(2)
            ]
            # normalize is split in two so the PE broadcast matmul of head h-1
            # lands between head h's S blocks (PE never waits on the DVE
            # reciprocal): row 32 of ot is the softmax denominator.
            pending = None  # (ot, recip, outT destination slice)

            def norm_finish():
                ot_p, recip, dstsl = pending
                rb = sp.tile([32, NB], f32, tag="s", name="rb")
                for fs0, fsn in splits:
                    nc.tensor.matmul(
                        rb[:, fs0 : fs0 + fsn],
                        ones32[0:1, :],
                        recip[:, fs0 : fs0 + fsn],
                        start=True,
                        stop=True,
                    )
                rb_sb = pxp.tile([32, NB], f32, tag="rb_sb")
                nc.vector.tensor_copy(rb_sb[:], rb[:])
                nc.vector.tensor_mul(dstsl, ot_p[0:32, :], rb_sb[:])

            for h in range(H):
                hi, hr = h // 4, (h % 4) * 32
                tpos = {"tile_position": (hr, 0)} if hr == 96 else {}
                ot = op.tile([33, NB], f32, tag="ot")
                for cc in range(NCH):
                    s_t = sp.tile([128, NB], f32, tag="s")
                    for fs0, fsn in splits:
                        nc.tensor.matmul(
                            s_t[:, fs0 : fs0 + fsn],
                            qk_slice("k", h)[:, cc * 128 : (cc + 1) * 128],
                            qk_slice("q", h)[:, fs0 : fs0 + fsn],
                            start=True,
                            stop=True,
                            **tpos,
                        )
                    # scatter matmuls: each (range, chunk) streams only its
                    # row range, clipped at the PSUM bank boundary; groups
                    # key on the exact region, so close each region's last op
                    bias_ops = []
                    for j, (col0, wn) in enumerate(ranges):
                        for ech in range(ECH):
                            for a, ln in subsplits(col0, wn):
                                bias_ops.append((j, ech, col0, a, ln))
                    last_in_region = {}
                    for oi, (j, ech, col0, a, ln) in enumerate(bias_ops):
                        last_in_region[(a, ln)] = oi
                    stops = set(last_in_region.values())
                    for oi, (j, ech, col0, a, ln) in enumerate(bias_ops):
                        nc.tensor.matmul(
                            s_t[:, a : a + ln],
                            sc8[(cc, j, ech)][:, h, :],
                            ohr[(cc, j, ech)][:, a - col0 : a - col0 + ln],
                            start=False,
                            stop=(oi in stops),
                            skip_group_check=True,
                        )
                    p_t = pxp.tile([128, NB], f16, tag="p")
                    nc.scalar.activation(
                        p_t[:],
                        s_t[:],
                        mybir.ActivationFunctionType.Exp,
                        bias=maskt[:, cc : cc + 1],
                        scale=1.0,
                    )
                    for fs0, fsn in splits:
                        nc.tensor.matmul(
                            ot[:, fs0 : fs0 + fsn],
                            v33[cc][:, h, :],
                            p_t[:, fs0 : fs0 + fsn],
                            start=(cc == 0),
                            stop=(cc == NCH - 1),
                        )
                    if cc == 3 and pending is not None:
                        norm_finish()
                        pending = None
                recip = pxp.tile([1, NB], f32, tag="recip")
                nc.vector.reciprocal(recip[:], ot[32:33, :])
                pending = (ot, recip, outT[hi][hr : hr + 32, :])
            norm_finish()

            # ---- final projection y^T = Wo'^T @ [outT; 1] ----
            out_k = [outT[0], outT[1], ones16]
            for mg in range(2):
                dst = pp.tile([128, NB], f16, tag=f"yt{mg}", name=f"yts{mg}")
                acc = sp.tile([128, NB], f32, tag="s", name="acc")
                for fs0, fsn in splits:
                    for ki in range(NKI):
                        nc.tensor.matmul(
                            acc[:, fs0 : fs0 + fsn],
                            wt["wo"][ki][:, mg * 128 : (mg + 1) * 128],
                            out_k[ki][:, fs0 : fs0 + fsn]
                            if ki < 2
                            else ones16[0:1, fs0 : fs0 + fsn],
                            start=(ki == 0),
                            stop=(ki == NKI - 1),
                        )
                nc.vector.tensor_copy(dst[:], acc[:])
                nc.sync.dma_start(out=yt_d[mg * 128 : (mg + 1) * 128, :], in_=dst[:])

    return nc


def kernel(x, edge_index, edge_attr, batch, Wq, bq, Wk, bk, Wv, bv, Wo, bo, We, be):
    x = np.asarray(x, np.float32)
    edge_index = np.asarray(edge_index)
    edge_attr = np.asarray(edge_attr, np.float32)
    batch = np.asarray(batch)
    n = x.shape[0]

    counts = np.bincount(batch.astype(np.int64), minlength=NCORES)
    starts = np.concatenate([[0], np.cumsum(counts)])[:NCORES]
    NB = max(640, int(-(-counts.max() // 128)) * 128)
    NCH = NB // 128

    # edge bias values; bucket same-graph edges by (key-column chunk,
    # query-row range) so the scatter matmuls only stream their row range
    eb = edge_attr @ np.asarray(We, np.float32) + np.asarray(be, np.float32)  # [E,H]
    r_all, c_all = edge_index[0], edge_index[1]
    br, bc = batch[r_all], batch[c_all]

    ranges = _ranges_for(NB)
    NRR = len(ranges)
    buckets = {}
    maxlen = 1
    for b in range(NCORES):
        s0 = int(starts[b])
        sel = np.where((br == b) & (bc == b))[0]
        rl = (r_all[sel] - s0).astype(np.int64)
        cl = (c_all[sel] - s0).astype(np.int64)
        for cc in range(NCH):
            for j, (r0, rw) in enumerate(ranges):
                m = (
                    (cl >= cc * 128)
                    & (cl < (cc + 1) * 128)
                    & (rl >= r0)
                    & (rl < r0 + rw)
                )
                idx = sel[m]
                buckets[(b, cc, j)] = (
                    (cl[m] - cc * 128).astype(np.float16),
                    rl[m].astype(np.float16),
                    eb[idx].astype(np.float16),
                )
                maxlen = max(maxlen, len(idx))
    CAP = max(128, int(-(-maxlen // 128)) * 128)

    # augmented weights [257, 256] packed as [514, 128]
    wpacks = []
    for W, bvec in ((Wq, bq), (Wk, bk), (Wv, bv), (Wo, bo)):
        wa = np.vstack([np.asarray(W, np.float32), np.asarray(bvec, np.float32)[None]])
        wpacks.append(wa.astype(np.float16).reshape(257, 2, 128).reshape(514, 128))

    nslot = NCH * NRR
    nblk = -(-nslot // SLOTS_PER_BLK)
    HB = int(
        any(np.any(np.asarray(v)) for v in (bq, bk, bv, bo))
    )
    in_maps = []
    for b in range(NCORES):
        s0, nb = int(starts[b]), int(counts[b])
        xT = np.zeros((256, NB), np.float16)
        xT[:, :nb] = x[s0 : s0 + nb].T
        xt_pack = xT.reshape(256, NCH, 128).reshape(256 * NCH, 128)

        eblk = np.zeros((nblk, CAP, 128), np.float16)
        for cc in range(NCH):
            for j in range(NRR):
                slot = cc * NRR + j
                blk, w0 = slot // SLOTS_PER_BLK, (slot % SLOTS_PER_BLK) * 10
                clb, rlb, ebb = buckets[(b, cc, j)]
                ne = len(clb)
                eblk[blk, :ne, w0] = clb
                eblk[blk, :ne, w0 + 1] = rlb
                eblk[blk, :ne, w0 + 2 : w0 + 10] = ebb
        # column mask block [128, NCH]
        mblk = np.zeros((128, 128), np.float16)
        gidx = np.arange(128)[:, None] + 128 * np.arange(NCH)[None, :]
        mblk[:, :NCH] = np.where(gidx < nb, MSK_VALID, MSK_PAD).astype(np.float16)

        inp = np.concatenate(
            [xt_pack] + wpacks + [eblk.reshape(nblk * CAP, 128), mblk], axis=0
        )
        in_maps.append({"inp": inp})

    key = (NB, CAP, HB)
    if key not in _prog_cache:
        _prog_cache[key] = _build_program(NB, CAP, HB)
    nc = _prog_cache[key]

    global _last_in_maps
    _last_in_maps = in_maps
    res = run_bass_kernel_spmd(nc, in_maps, list(range(NCORES)))
    y = np.empty((n, D), np.float32)
    for b in range(NCORES):
        s0, nb = int(starts[b]), int(counts[b])
        y[s0 : s0 + nb] = res.results[b]["yt"][:, :nb].T.astype(np.float32)
    return y


# revision 38
# speedup vs baseline: 1.1798x; 1.1798x over previous
"""Graphormer multi-head attention on 8 trn2 NeuronCores.

Sharding: sequence-parallel over the 8 sorted batch segments (one graph
per core). Each core runs dense block attention for all 8 heads over its
~512-node segment, padded to a common NB so the program is SPMD.

I/O design: the per-call cost in this environment is dominated by input
marshaling (per-buffer and per-byte), so the program takes ONE packed
fp16 input per core (~1MB: x^T slice, augmented projection weights,
bucketed edge-bias COO data, column mask) and returns one fp16 output.
The edge bias is scattered on-device: scaled one-hot matrices are built
with iota + tensor_scalar(is_equal, mult) and injected into the score
PSUM accumulation as extra rank-|bucket| matmuls.

Math (transposed so the softmax reduction rides the matmul contraction):
  S^T[c, r] = K[c, :] . Q[r, :] / sqrt(HD)     (PE, fp16 operands)
  S^T      += sum_e onehotC_e(c) eb_e onehotR_e(r)   (PE, per c-chunk bucket)
  P = exp(S^T + colmask - 6)                   (ACT; -6 keeps exp in fp16)
  OT'[d, r] = sum_c V'[c, d] P[c, r]           (PE; V' has a ones column ->
                                                row 32 of OT' = denominator)
  outT = OT'[0:32] * bcast(1/den)              (DVE; bcast via K=1 outer product)
  y^T = Wo'^T @ [outT; 1]                      (PE; bias via augmented ones row)
"""

import sys

for _p in ("/opt/trn_rl_repo",):
    if _p not in sys.path:
        sys.path.insert(0, _p)

import numpy as np

import concourse.bass as bass
import concourse.mybir as mybir
import concourse.tile as tile
from concourse.bass_utils import run_bass_kernel_spmd

N, D, H, HD, NCORES = 4096, 256, 8, 32, 8

# ---------------------------------------------------------------------------
# This toolchain's CoreV3 codegen accepts at most ONE semaphore wait per
# engine instruction ("Too many sync wait commands").  Tile freely emits
# several.  Engine queues execute in order, so it is equivalent to hoist all
# but one wait onto single-wait NoOps inserted immediately before the
# instruction on the same engine.  Do that as a BIR-JSON rewrite just before
# neuronxcc compilation.
import json as _json

import concourse.bass2jax as _b2j

_SKIP_OPS = {"EventSemaphore", "UnconditionalBranch", "ConditionalBranch"}


def _split_multiwaits(bir_json: bytes) -> bytes:
    d = _json.loads(bir_json)
    nid = [0]
    for fn in d.get("functions", []):
        for blk in fn.get("blocks", []):
            out = []
            for inst in blk.get("instructions", []):
                si = inst.get("sync_info")
                ow = (si or {}).get("on_wait") or []
                if len(ow) > 1 and inst.get("opcode") not in _SKIP_OPS:
                    for w in ow[:-1]:
                        nid[0] += 1
                        out.append(
                            {
                                "debug": inst.get("debug", 0),
                                "engine": inst["engine"],
                                "ins": [],
                                "name": f"I-waitsplit-{nid[0]}",
                                "opcode": "NoOp",
                                "outs": [],
                                "sync_info": {"on_update": [], "on_wait": [w]},
                            }
                        )
                    si["on_wait"] = [ow[-1]]
                out.append(inst)
            blk["instructions"] = out
    return _json.dumps(d).encode()


_orig_cbk = _b2j.compile_bir_kernel


def _cbk(bir_json, tmpdir, neff_name="file.neff"):
    return _orig_cbk(_split_multiwaits(bir_json), tmpdir, neff_name=neff_name)


if getattr(_b2j.compile_bir_kernel, "__name__", "") != "_cbk":
    _b2j.compile_bir_kernel = _cbk

SCALE = 1.0 / np.sqrt(HD)
MSK_VALID = -6.0  # constant shift: cancels in softmax, keeps exp(.) in fp16
MSK_PAD = -60000.0  # exp -> 0 for padded key columns
NR = 4  # query-row ranges per bucket (scatter matmuls stream only their range)
SLOTS_PER_BLK = 12  # (cc, range) slots of 10 cols per 128-wide edge block

_prog_cache = {}
_last_in_maps = None


def _ranges_for(NB):
    # row ranges that never cross a 512-wide PSUM bank boundary: three
    # 32-aligned ranges over [0, 512), then one per 512 chunk beyond
    out = [(0, 160), (160, 176), (336, 176)]
    s = 512
    while s < NB:
        out.append((s, min(512, NB - s)))
        s += 512
    return out


def _build_program(NB, CAP, HB):
    NCH = NB // 128
    ECH = CAP // 128
    splits = [(s, min(512, NB - s)) for s in range(0, NB, 512)]
    ranges = _ranges_for(NB)
    NRR = len(ranges)
    NBLK = -(-(NCH * NRR) // SLOTS_PER_BLK)
    NKI = 3 if HB else 2  # drop the rank-1 bias aug chunk when biases are 0

    def subsplits(col0, wn):
        # intersect a row range with the 512-wide PSUM bank splits
        out = []
        for fs0, fsn in splits:
            a, bnd = max(col0, fs0), min(col0 + wn, fs0 + fsn)
            if a < bnd:
                out.append((a, bnd - a))
        return out

    f32 = mybir.dt.float32
    f16 = mybir.dt.float16

    # packed input row offsets (rows of 128 fp16)
    XT0 = 0
    W0 = XT0 + 256 * NCH
    E0 = W0 + 4 * 514
    M0 = E0 + NBLK * CAP
    RTOT = M0 + 128

    nc = bass.Bass()
    inp_d = nc.declare_dram_parameter("inp", [RTOT, 128], f16, isOutput=False)
    yt_d = nc.declare_dram_parameter("yt", [256, NB], f16, isOutput=True)

    kch = [(0, 128), (128, 128), (256, 1)]  # contraction chunks of the 257-row aug

    with tile.TileContext(nc) as tc:
        with (
            tc.tile_pool(name="persist", bufs=1) as pp,
            tc.tile_pool(name="pexp", bufs=4) as pxp,
            tc.tile_pool(name="ps_s", bufs=2, space="PSUM") as sp,
            tc.tile_pool(name="ps_o", bufs=2, space="PSUM") as op,
        ):
            # ---- load persistent operands from the packed input ----
            # HWDGE (sync) carries the big x/weight loads; SWDGE (gpsimd)
            # carries the cast loads; scalar (HWDGE) the tiny bias rows.
            iota = pp.tile([128, NB], f16, tag="iota")
            nc.gpsimd.iota(
                iota[:],
                pattern=[[1, NB]],
                base=0,
                channel_multiplier=0,
                allow_small_or_imprecise_dtypes=True,
            )
            # iota along c, replicated over the trailing head axis; the
            # [e, c, h] layout keeps every last dim stride-1 so the one-hot
            # builds qualify for the DVE 2x fp16 mode
            iota8 = pp.tile([128, 128, 8], f16, tag="iota8")
            nc.gpsimd.iota(
                iota8[:],
                pattern=[[1, 128], [0, 8]],
                base=0,
                channel_multiplier=0,
                allow_small_or_imprecise_dtypes=True,
            )
            xt = []
            for kc in range(2):
                t = pp.tile([128, NB], f16, tag=f"xt{kc}", name=f"xt{kc}")
                nc.gpsimd.dma_start(
                    out=t[:],
                    in_=inp_d[
                        XT0 + kc * 128 * NCH : XT0 + (kc + 1) * 128 * NCH, :
                    ].rearrange("(p f) w -> p (f w)", f=NCH),
                )
                xt.append(t)
            ones16 = pp.tile([1, NB], f16, tag="ones16")
            nc.vector.memset(ones16[:], 1.0)
            xt.append(ones16)  # aug ones row, contraction chunk (256, 1)

            # PE warm-up: the HAM clock gate releases only after ~3.4us of
            # sustained matmul activity. While the weight DMAs land, run free
            # dummy matmuls so the real stream starts at full clock.
            warm = sp.tile([128, 512], f32, tag="s", name="warm")
            for _ in range(16):
                nc.tensor.matmul(
                    warm[:],
                    ones16[0:1, 0:128],
                    ones16[0:1, 0:512],
                    start=True,
                    stop=True,
                )

            # edge blocks first on the scalar queue so the one-hot builds can
            # start immediately: [128, ECH, 128] fp16 per block (indices
            # ≤ 2048 are exact in fp16, so comparisons stay exact)
            ed_blk = []
            for blk in range(NBLK):
                t = pp.tile([128, ECH, 128], f16, tag=f"ed{blk}")
                nc.sync.dma_start(
                    out=t[:],
                    in_=inp_d[E0 + blk * CAP : E0 + (blk + 1) * CAP, :].rearrange(
                        "(e p) w -> p e w", p=128
                    ),
                )
                ed_blk.append(t)
            wt = {}
            for wi, nm in enumerate(("wq", "wk", "wv", "wo")):
                base = W0 + wi * 514
                w2 = pp.tile([128, 2, 256], f16, tag=f"{nm}w2", name=f"{nm}w2")
                nc.sync.dma_start(
                    out=w2[:],
                    in_=inp_d[base : base + 512, :].rearrange(
                        "(kc p h) w -> p kc (h w)", kc=2, p=128, h=2
                    ),
                )
                wt[nm] = [w2[:, 0, :], w2[:, 1, :]]
                if HB:
                    wb = pp.tile([1, 256], f16, tag=f"{nm}wb", name=f"{nm}wb")
                    nc.scalar.dma_start(
                        out=wb[:],
                        in_=inp_d[base + 512 : base + 514, :].rearrange(
                            "(p h) w -> p (h w)", p=1
                        ),
                    )
                    wt[nm].append(wb[:])

            maskt = pp.tile([128, NCH], f16, tag="mask")
            nc.sync.dma_start(out=maskt[:], in_=inp_d[M0 : M0 + 128, 0:NCH])

            # f32 view of the index columns (is_equal requires an f32 scalar)
            ed32_blk = []
            for blk in range(NBLK):
                t = pp.tile([128, ECH, 128], f32, tag=f"ed32_{blk}")
                nc.vector.tensor_copy(t[:], ed_blk[blk][:])
                ed32_blk.append(t)

            def _slot(cc, j):
                slot = cc * NRR + j
                return slot // SLOTS_PER_BLK, (slot % SLOTS_PER_BLK) * 10

            def ed_cl(cc, j, ech):
                blk, w0 = _slot(cc, j)
                return ed32_blk[blk][:, ech, w0 : w0 + 1]

            def ed_rl(cc, j, ech):
                blk, w0 = _slot(cc, j)
                return ed32_blk[blk][:, ech, w0 + 1 : w0 + 2]

            def ed_eb(cc, j, ech):
                blk, w0 = _slot(cc, j)
                return ed_blk[blk][:, ech, w0 + 2 : w0 + 10]

            ones32 = pp.tile([1, 32], f32, tag="ones32")
            nc.vector.memset(ones32[:], 1.0)

            # ---- one-hot selectors per (key-chunk, row-range, edge chunk) --
            # ohr: row one-hots [edges, range] (only the range's columns);
            # sc8: per-head scaled col one-hots [edges, 8, 128]
            # = (iota == cl) * eb_h.  oh8 alternates DVE/GPSIMD for balance.
            ohr = {}
            sc8 = {}
            for cc in range(NCH):
                for j, (col0, wn) in enumerate(ranges):
                    for ech in range(ECH):
                        t = pp.tile([128, wn], f16, tag=f"ohr{cc}_{j}_{ech}")
                        nc.vector.tensor_scalar(
                            t[:],
                            iota[:, col0 : col0 + wn],
                            ed_rl(cc, j, ech),
                            None,
                            mybir.AluOpType.is_equal,
                        )
                        ohr[(cc, j, ech)] = t
                        oh8 = pxp.tile([128, 128, 8], f16, tag="oh8")
                        beng = nc.vector if (cc * NRR + j) % 2 else nc.gpsimd
                        beng.tensor_scalar(
                            oh8[:],
                            iota8[:],
                            ed_cl(cc, j, ech),
                            None,
                            mybir.AluOpType.is_equal,
                        )
                        s8 = pp.tile([128, 128, 8], f16, tag=f"sc8{cc}_{j}_{ech}")
                        nc.vector.tensor_tensor(
                            s8[:],
                            oh8[:],
                            ed_eb(cc, j, ech)
                            .unsqueeze(1)
                            .to_broadcast((128, 128, 8)),
                            mybir.AluOpType.mult,
                        )
                        sc8[(cc, j, ech)] = s8

            # ---- Q^T, K^T: two groups of 4 heads; head slice base partition
            # 0/32/64 auto, 96 via explicit tile_position.
            qk_tiles = {}
            for key in ("q", "k"):
                qk_tiles[key] = [
                    pp.tile([128, NB], f16, tag=f"{key}g{g}", name=f"{key}g{g}")
                    for g in range(2)
                ]

            def qk_slice(key, h):
                return qk_tiles[key][h // 4][(h % 4) * 32 : (h % 4) * 32 + 32]

            for nm, key, scl in (("wq", "q", SCALE), ("wk", "k", 1.0)):
                for mg in range(2):
                    acc = sp.tile([128, NB], f32, tag="s", name="acc")
                    for fs0, fsn in splits:
                        for ki in range(NKI):
                            nc.tensor.matmul(
                                acc[:, fs0 : fs0 + fsn],
                                wt[nm][ki][:, mg * 128 : (mg + 1) * 128],
                                xt[ki][:, fs0 : fs0 + fsn],
                                start=(ki == 0),
                                stop=(ki == NKI - 1),
                            )
                    nc.scalar.activation(
                        qk_tiles[key][mg][:],
                        acc[:],
                        mybir.ActivationFunctionType.Copy,
                        scale=scl,
                    )

            # ---- V natural layout, per 128-row chunk, with ones column ----
            v33 = []
            for rc in range(NCH):
                dst = pp.tile([128, 8, 33], f16, tag=f"v33_{rc}")
                acc = sp.tile([128, NB], f32, tag="s", name="accv")[:, 0:256].rearrange("p (h d) -> p h d", h=8)
                for ki in range(NKI):
                    nc.tensor.matmul(
                        acc[:],
                        xt[ki][:, rc * 128 : (rc + 1) * 128],
                        wt["wv"][ki][:],
                        start=(ki == 0),
                        stop=(ki == NKI - 1),
                    )
                nc.scalar.activation(
                    dst[:, :, 0:32], acc[:], mybir.ActivationFunctionType.Copy
                )
                nc.vector.memset(dst[:, :, 32:33], 1.0)
                v33.append(dst)

            # ---- attention per head ----
            outT = [
                pp.tile([128, NB], f16, tag=f"outT{mg}", name=f"outT{mg}")
                for mg in range(2)
            ]
            # normalize is split in two so the PE broadcast matmul of head h-1
            # lands between head h's S blocks (PE never waits on the DVE
            # reciprocal): row 32 of ot is the softmax denominator.
            pending = None  # (ot, recip, outT destination slice)

            def norm_finish():
                ot_p, recip, dstsl = pending
                rb = sp.tile([32, NB], f32, tag="s", name="rb")
                for fs0, fsn in splits:
                    nc.tensor.matmul(
                        rb[:, fs0 : fs0 + fsn],
                        ones32[0:1, :],
                        recip[:, fs0 : fs0 + fsn],
                        start=True,
                        stop=True,
                    )
                rb_sb = pxp.tile([32, NB], f32, tag="rb_sb")
                nc.vector.tensor_copy(rb_sb[:], rb[:])
                nc.vector.tensor_mul(dstsl, ot_p[0:32, :], rb_sb[:])

            for h in range(H):
                hi, hr = h // 4, (h % 4) * 32
                tpos = {"tile_position": (hr, 0)} if hr == 96 else {}
                ot = op.tile([33, NB], f32, tag="ot")
                for cc in range(NCH):
                    s_t = sp.tile([128, NB], f32, tag="s")
                    for fs0, fsn in splits:
                        nc.tensor.matmul(
                            s_t[:, fs0 : fs0 + fsn],
                            qk_slice("k", h)[:, cc * 128 : (cc + 1) * 128],
                            qk_slice("q", h)[:, fs0 : fs0 + fsn],
                            start=True,
                            stop=True,
                            **tpos,
                        )
                    # scatter matmuls: each (range, chunk) streams only its
                    # row range, clipped at the PSUM bank boundary; groups
                    # key on the exact region, so close each region's last op
                    bias_ops = []
                    for j, (col0, wn) in enumerate(ranges):
                        for ech in range(ECH):
                            for a, ln in subsplits(col0, wn):
                                bias_ops.append((j, ech, col0, a, ln))
                    last_in_region = {}
                    for oi, (j, ech, col0, a, ln) in enumerate(bias_ops):
                        last_in_region[(a, ln)] = oi
                    stops = set(last_in_region.values())
                    for oi, (j, ech, col0, a, ln) in enumerate(bias_ops):
                        nc.tensor.matmul(
                            s_t[:, a : a + ln],
                            sc8[(cc, j, ech)][:, :, h],
                            ohr[(cc, j, ech)][:, a - col0 : a - col0 + ln],
                            start=False,
                            stop=(oi in stops),
                            skip_group_check=True,
                        )
                    p_t = pxp.tile([128, NB], f16, tag="p")
                    nc.scalar.activation(
                        p_t[:],
                        s_t[:],
                        mybir.ActivationFunctionType.Exp,
                        bias=maskt[:, cc : cc + 1],
                        scale=1.0,
                    )
                    for fs0, fsn in splits:
                        nc.tensor.matmul(
                            ot[:, fs0 : fs0 + fsn],
                            v33[cc][:, h, :],
                            p_t[:, fs0 : fs0 + fsn],
                            start=(cc == 0),
                            stop=(cc == NCH - 1),
                        )
                    if cc == 3 and pending is not None:
                        norm_finish()
                        pending = None
                recip = pxp.tile([1, NB], f32, tag="recip")
                nc.vector.reciprocal(recip[:], ot[32:33, :])
                pending = (ot, recip, outT[hi][hr : hr + 32, :])
            norm_finish()

            # ---- final projection y^T = Wo'^T @ [outT; 1] ----
            out_k = [outT[0], outT[1], ones16]
            for mg in range(2):
                dst = pp.tile([128, NB], f16, tag=f"yt{mg}", name=f"yts{mg}")
                acc = sp.tile([128, NB], f32, tag="s", name="acc")
                for fs0, fsn in splits:
                    for ki in range(NKI):
                        nc.tensor.matmul(
                            acc[:, fs0 : fs0 + fsn],
                            wt["wo"][ki][:, mg * 128 : (mg + 1) * 128],
                            out_k[ki][:, fs0 : fs0 + fsn]
                            if ki < 2
                            else ones16[0:1, fs0 : fs0 + fsn],
                            start=(ki == 0),
                            stop=(ki == NKI - 1),
                        )
                nc.vector.tensor_copy(dst[:], acc[:])
                nc.sync.dma_start(out=yt_d[mg * 128 : (mg + 1) * 128, :], in_=dst[:])

    return nc


def kernel(x, edge_index, edge_attr, batch, Wq, bq, Wk, bk, Wv, bv, Wo, bo, We, be):
    x = np.asarray(x, np.float32)
    edge_index = np.asarray(edge_index)
    edge_attr = np.asarray(edge_attr, np.float32)
    batch = np.asarray(batch)
    n = x.shape[0]

    counts = np.bincount(batch.astype(np.int64), minlength=NCORES)
    starts = np.concatenate([[0], np.cumsum(counts)])[:NCORES]
    NB = max(640, int(-(-counts.max() // 128)) * 128)
    NCH = NB // 128

    # edge bias values; bucket same-graph edges by (key-column chunk,
    # query-row range) so the scatter matmuls only stream their row range
    eb = edge_attr @ np.asarray(We, np.float32) + np.asarray(be, np.float32)  # [E,H]
    r_all, c_all = edge_index[0], edge_index[1]
    br, bc = batch[r_all], batch[c_all]

    ranges = _ranges_for(NB)
    NRR = len(ranges)
    buckets = {}
    maxlen = 1
    for b in range(NCORES):
        s0 = int(starts[b])
        sel = np.where((br == b) & (bc == b))[0]
        rl = (r_all[sel] - s0).astype(np.int64)
        cl = (c_all[sel] - s0).astype(np.int64)
        for cc in range(NCH):
            for j, (r0, rw) in enumerate(ranges):
                m = (
                    (cl >= cc * 128)
                    & (cl < (cc + 1) * 128)
                    & (rl >= r0)
                    & (rl < r0 + rw)
                )
                idx = sel[m]
                buckets[(b, cc, j)] = (
                    (cl[m] - cc * 128).astype(np.float16),
                    rl[m].astype(np.float16),
                    eb[idx].astype(np.float16),
                )
                maxlen = max(maxlen, len(idx))
    CAP = max(128, int(-(-maxlen // 128)) * 128)

    # augmented weights [257, 256] packed as [514, 128]
    wpacks = []
    for W, bvec in ((Wq, bq), (Wk, bk), (Wv, bv), (Wo, bo)):
        wa = np.vstack([np.asarray(W, np.float32), np.asarray(bvec, np.float32)[None]])
        wpacks.append(wa.astype(np.float16).reshape(257, 2, 128).reshape(514, 128))

    nslot = NCH * NRR
    nblk = -(-nslot // SLOTS_PER_BLK)
    HB = int(
        any(np.any(np.asarray(v)) for v in (bq, bk, bv, bo))
    )
    in_maps = []
    for b in range(NCORES):
        s0, nb = int(starts[b]), int(counts[b])
        xT = np.zeros((256, NB), np.float16)
        xT[:, :nb] = x[s0 : s0 + nb].T
        xt_pack = xT.reshape(256, NCH, 128).reshape(256 * NCH, 128)

        eblk = np.zeros((nblk, CAP, 128), np.float16)
        for cc in range(NCH):
            for j in range(NRR):
                slot = cc * NRR + j
                blk, w0 = slot // SLOTS_PER_BLK, (slot % SLOTS_PER_BLK) * 10
                clb, rlb, ebb = buckets[(b, cc, j)]
                ne = len(clb)
                eblk[blk, :ne, w0] = clb
                eblk[blk, :ne, w0 + 1] = rlb
                eblk[blk, :ne, w0 + 2 : w0 + 10] = ebb
        # column mask block [128, NCH]
        mblk = np.zeros((128, 128), np.float16)
        gidx = np.arange(128)[:, None] + 128 * np.arange(NCH)[None, :]
        mblk[:, :NCH] = np.where(gidx < nb, MSK_VALID, MSK_PAD).astype(np.float16)

        inp = np.concatenate(
            [xt_pack] + wpacks + [eblk.reshape(nblk * CAP, 128), mblk], axis=0
        )
        in_maps.append({"inp": inp})

    key = (NB, CAP, HB)
    if key not in _prog_cache:
        _prog_cache[key] = _build_program(NB, CAP, HB)
    nc = _prog_cache[key]

    global _last_in_maps
    _last_in_maps = in_maps
    res = run_bass_kernel_spmd(nc, in_maps, list(range(NCORES)))
    y = np.empty((n, D), np.float32)
    for b in range(NCORES):
        s0, nb = int(starts[b]), int(counts[b])
        y[s0 : s0 + nb] = res.results[b]["yt"][:, :nb].T.astype(np.float32)
    return y
